# revision 7
# baseline (speedup 1.0000x reference)
"""Trainium2 Bass kernel for nn_DecoderLayer_86036784873883.

Data-parallel over tokens: each of 8 cores handles 512 contiguous tokens.
The torch-faithful reshape q.view(B,H,S,D) with no transpose makes each
"head" (b,h) attend only within a contiguous 64-original-token block, so a
contiguous token shard keeps attention fully core-local (no collectives).

Per-core layouts (E=2048 features split into 16 blocks of 128):
  xT/hT/h2T/o_fullT: [128 part=feat-in-block, 16 block, 512 token]
  qT/kT: same, feature-major (feeds PE contraction over head_dim=128)
  v: token-major [128 tok, 2048 feat], then per-unit DMA-rearranged to
     [128 = (eblk-parity,tok), 8 tc-chunk, 128 d] for the AV matmul.
  Attention seq axis is permuted e-major (sc' = eblk*64 + i); the causal
  mask is precomputed on host in that coordinate system (additive, raw
  scale, bf16) and added into the scores PSUM via an identity matmul.
  MoE computed dense (all 8 experts, matching the reference math) as one
  FF=8192 MLP with per-(token, expert) hidden scaling by top-2 combine
  weights. Matmuls use float32r (TF32-like, full PE rate at N>=512).
"""

import sys
sys.path.insert(0, '/opt/trn_rl_repo')
import numpy as np
import concourse.bacc as bacc
import concourse.tile as tile
import concourse.mybir as mybir
from concourse.bass_utils import run_bass_kernel_spmd

F32 = mybir.dt.float32
F32R = mybir.dt.float32r
BF16 = mybir.dt.bfloat16
AF = mybir.ActivationFunctionType
AL = mybir.AluOpType

B, S, E = 4, 1024, 2048
N_CORES = 8
NTOK = B * S // N_CORES          # 512 tokens per core
NB = E // 128                    # 16 feature blocks
HID = 8192                       # dense MoE hidden (8 experts x 1024)
NEXP = 8
SCALING = float(128) ** 0.5
MASK_NEG = -10000.0 * SCALING    # additive mask before 1/scaling is applied
EPS = float(np.finfo(np.float32).eps)

_CACHED = {}


def _build():
    nc = bacc.Bacc()
    # ---- DRAM I/O ----
    xT = nc.dram_tensor("xT", [E, NTOK], F32, kind="ExternalInput")
    qkv_WT = nc.dram_tensor("qkv_WT", [E, 3 * E], F32R, kind="ExternalInput")
    out_WT = nc.dram_tensor("out_WT", [E, E], F32R, kind="ExternalInput")
    w1T = nc.dram_tensor("w1T", [E, HID], F32R, kind="ExternalInput")
    w2T = nc.dram_tensor("w2T", [HID, E], F32R, kind="ExternalInput")
    gate_WT = nc.dram_tensor("gate_WT", [E, NEXP], F32R, kind="ExternalInput")
    n1w = nc.dram_tensor("n1w", [128, NB], F32, kind="ExternalInput")
    n2w = nc.dram_tensor("n2w", [128, NB], F32, kind="ExternalInput")
    maskT = nc.dram_tensor("maskT", [S, S], BF16, kind="ExternalInput")
    identf = nc.dram_tensor("identf", [128, 128], F32, kind="ExternalInput")

    outT = nc.dram_tensor("outT", [E, NTOK], F32, kind="ExternalOutput")
    probs8 = nc.dram_tensor("probs8", [1, NEXP], F32, kind="ExternalOutput")

    with tile.TileContext(nc) as tc:
        import contextlib
        ctx = contextlib.ExitStack()
        with ctx:
            big = ctx.enter_context(tc.tile_pool(name="big", bufs=1))
            wst = ctx.enter_context(tc.tile_pool(name="wst", bufs=5))
            tmp = ctx.enter_context(tc.tile_pool(name="tmp", bufs=3))
            pTp = ctx.enter_context(tc.tile_pool(name="pTp", bufs=9))
            sml = ctx.enter_context(tc.tile_pool(name="sml", bufs=1))
            rcb = ctx.enter_context(tc.tile_pool(name="rcb", bufs=2))
            one = ctx.enter_context(tc.tile_pool(name="one", bufs=1))
            psM = ctx.enter_context(tc.tile_pool(name="psM", bufs=4, space="PSUM"))
            psA = ctx.enter_context(tc.tile_pool(name="psA", bufs=2, space="PSUM"))
            psS = ctx.enter_context(tc.tile_pool(name="psS", bufs=1, space="PSUM"))
            psB = ctx.enter_context(tc.tile_pool(name="psB", bufs=1, space="PSUM"))

            # ---- constants ----
            id_f = one.tile([128, 128], F32)
            nc.sync.dma_start(out=id_f, in_=identf[:, :])
            id_bf = one.tile([128, 128], BF16)
            nc.vector.tensor_copy(id_bf, id_f)
            ones_row_f = one.tile([1, 128], F32)      # K=1 broadcast lhsT
            nc.vector.memset(ones_row_f, 1.0)
            ones_col_f = one.tile([128, 1], F32)      # K=128 sum lhsT (fp32)
            nc.vector.memset(ones_col_f, 1.0)
            ones_col_bf = one.tile([128, 1], BF16)
            nc.vector.tensor_copy(ones_col_bf, ones_col_f)
            ones_col_r = one.tile([128, 1], F32R)
            nc.vector.tensor_copy(ones_col_r, ones_col_f)
            eps_t = one.tile([1, 1], F32)
            nc.vector.memset(eps_t, EPS)
            n1w_sb = one.tile([128, NB], F32)
            nc.sync.dma_start(out=n1w_sb, in_=n1w[:, :])
            n2w_sb = one.tile([128, NB], F32)
            nc.sync.dma_start(out=n2w_sb, in_=n2w[:, :])
            gwt_sb = one.tile([128, NB, NEXP], F32R)
            nc.sync.dma_start(out=gwt_sb, in_=gate_WT.rearrange("(c p) e -> p c e", p=128))
            mask_sb = one.tile([128, 8, S], BF16)
            nc.sync.dma_start(out=mask_sb, in_=maskT.rearrange("(c p) s -> p c s", p=128))

            # ---- load x (transposed on host) ----
            xT_sb = big.tile([128, NB, NTOK], F32, tag="bigA")
            nc.sync.dma_start(out=xT_sb, in_=xT.rearrange("(c p) n -> p c n", p=128))

            def rmsnorm(src, w_sb, dst_tag):
                """src [128, NB, NTOK] f32 -> normalized f32r tile (same shape)."""
                ssq = psS.tile([1, NTOK], F32, tag="sums")
                for c in range(NB):
                    x2 = tmp.tile([128, NTOK], F32R, tag="scr")
                    nc.vector.tensor_mul(x2, src[:, c, :], src[:, c, :])
                    nc.tensor.matmul(ssq, lhsT=ones_col_r, rhs=x2,
                                     start=(c == 0), stop=(c == NB - 1))
                std = sml.tile([1, NTOK], F32, tag="rowA")
                nc.scalar.activation(out=std, in_=ssq, func=AF.Sqrt,
                                     bias=eps_t, scale=1.0 / E)
                rstd = sml.tile([1, NTOK], F32, tag="rowB")
                nc.vector.reciprocal(rstd, std)
                rb_ps = psB.tile([128, NTOK], F32, tag="bcast")
                nc.tensor.matmul(rb_ps, lhsT=ones_row_f, rhs=rstd,
                                 start=True, stop=True)
                h_sb = big.tile([128, NB, NTOK], F32R, tag=dst_tag)
                for c in range(NB):
                    nc.vector.scalar_tensor_tensor(
                        out=h_sb[:, c, :], in0=src[:, c, :],
                        scalar=w_sb[:, c:c + 1], in1=rb_ps,
                        op0=AL.mult, op1=AL.mult)
                return h_sb

            hT = rmsnorm(xT_sb, n1w_sb, "bigB")

            # ---- qkv projections ----
            # unit-major: [d, unit, eblk, tok-in-unit] so per-unit matmul
            # operand slices collapse to a single contiguous free dim
            qT_sb = big.tile([128, 8, NB, 64], F32R, tag="bigC")
            kT_sb = big.tile([128, 8, NB, 64], F32R, tag="bigD")
            v_sb = big.tile([128, 4, E], BF16, tag="vsb")

            # q, k: feature-major (variant B), 8 jgroups of 4 x 128 features
            for part, store in ((0, qT_sb), (1, kT_sb)):
                for jg in range(4):
                    ps4 = [psM.tile([128, NTOK], F32, tag="main", name=f"psq{i}") for i in range(4)]
                    for ec in range(NB):
                        wt = wst.tile([128, 512], F32R, tag="wst")
                        col0 = part * E + jg * 512
                        eng = nc.sync if ec % 2 == 0 else nc.scalar
                        eng.dma_start(
                            out=wt, in_=qkv_WT[ec * 128:(ec + 1) * 128,
                                              col0:col0 + 512])
                        for q in range(4):
                            nc.tensor.matmul(ps4[q], lhsT=wt[:, q * 128:(q + 1) * 128],
                                             rhs=hT[:, ec, :],
                                             start=(ec == 0), stop=(ec == NB - 1))
                    for q in range(4):
                        nc.vector.tensor_copy(
                            store[:, :, jg * 4 + q, :],
                            ps4[q].rearrange("p (u i) -> p u i", u=8))

            # v: token-major (variant A): psum [tok 128, feat 512]
            for vg in range(4):
                ps4 = [psM.tile([128, 512], F32, tag="main", name=f"psv{i}") for i in range(4)]
                for ec in range(NB):
                    wt = wst.tile([128, 512], F32R, tag="wst")
                    col0 = 2 * E + vg * 512
                    eng = nc.sync if ec % 2 == 0 else nc.scalar
                    eng.dma_start(
                        out=wt, in_=qkv_WT[ec * 128:(ec + 1) * 128, col0:col0 + 512])
                    for tch in range(4):
                        nc.tensor.matmul(ps4[tch], lhsT=hT[:, ec, tch * 128:(tch + 1) * 128],
                                         rhs=wt, start=(ec == 0), stop=(ec == NB - 1))
                for tch in range(4):
                    nc.vector.tensor_copy(v_sb[:, tch, vg * 512:(vg + 1) * 512], ps4[tch])

            # per-unit v rearrange: v_u[b*64+i, t, d] = v[64u+i, (2t+b)*128+d]
            v_us = []
            for u in range(8):
                v_u = tmp.tile([128, 8, 128], BF16, tag="v_u")
                src = v_sb[:, u // 2, :].rearrange("p (t b d) -> p t b d", t=8, b=2)
                off = (u % 2) * 64
                for b in range(2):
                    nc.sync.dma_start(out=v_u[b * 64:(b + 1) * 64, :, :],
                                      in_=src[off:off + 64, :, b, :])
                v_us.append(v_u)

            # ---- attention (8 units; seq permuted e-major) ----
            o_sb = big.tile([128, NB, NTOK], F32R, tag="bigB")
            for u in range(8):
                for s in range(2):
                    av = psA.tile([128, 512], F32, tag="acc")
                    sums = psS.tile([1, 512], F32, tag="sums")
                    pts = []
                    for t in range(8):
                        sc_ps = psM.tile([128, 512], F32, tag="main")
                        nc.tensor.matmul(sc_ps, lhsT=id_bf,
                                         rhs=mask_sb[:, t, s * 512:(s + 1) * 512],
                                         start=True, stop=False)
                        nc.tensor.matmul(
                            sc_ps,
                            lhsT=kT_sb[:, u, 2 * t:2 * t + 2, :],
                            rhs=qT_sb[:, u, 8 * s:8 * s + 8, :],
                            start=False, stop=True)
                        pt = pTp.tile([128, 512], BF16, tag="pT")
                        nc.scalar.activation(out=pt, in_=sc_ps, func=AF.Exp,
                                             scale=1.0 / SCALING)
                        nc.tensor.matmul(sums, lhsT=ones_col_bf, rhs=pt,
                                         start=(t == 0), stop=(t == 7))
                        nc.tensor.matmul(av, lhsT=v_us[u][:, t, :], rhs=pt,
                                         start=(t == 0), stop=(t == 7))
                        pts.append(pt)
                    rec = sml.tile([1, 512], F32, tag="rowA")
                    nc.vector.reciprocal(rec, sums)
                    rb = psB.tile([128, 512], F32, tag="bcast")
                    nc.tensor.matmul(rb, lhsT=ones_row_f, rhs=rec, start=True, stop=True)
                    rec_sb = rcb.tile([128, 512], F32, tag="recb")
                    nc.vector.tensor_copy(rec_sb, rb)
                    for eb in range(8):
                        sl = slice(eb * 64, eb * 64 + 64)
                        nc.vector.tensor_mul(
                            o_sb[:, 8 * s + eb, 64 * u:64 * u + 64],
                            av[:, sl], rec_sb[:, sl])

            # ---- output projection + residual -> x1T (f32) ----
            x1T = big.tile([128, NB, NTOK], F32, tag="bigD")
            for jg in range(4):
                ps4 = [psM.tile([128, NTOK], F32, tag="main", name=f"psq{i}") for i in range(4)]
                for ec in range(NB):
                    wt = wst.tile([128, 512], F32R, tag="wst")
                    eng = nc.sync if ec % 2 == 0 else nc.scalar
                    eng.dma_start(
                        out=wt, in_=out_WT[ec * 128:(ec + 1) * 128,
                                           jg * 512:(jg + 1) * 512])
                    for q in range(4):
                        nc.tensor.matmul(ps4[q], lhsT=wt[:, q * 128:(q + 1) * 128],
                                         rhs=o_sb[:, ec, :],
                                         start=(ec == 0), stop=(ec == NB - 1))
                for q in range(4):
                    jc = jg * 4 + q
                    nc.vector.tensor_add(x1T[:, jc, :], ps4[q], xT_sb[:, jc, :])

            # ---- rmsnorm2 -> h2T ----
            h2T = rmsnorm(x1T, n2w_sb, "bigB")

            # ---- gating (token-major), top-2 combine weights ----
            cw_sb = one.tile([128, 4, NEXP], F32)
            p8_ps = psB.tile([1, NEXP], F32, tag="bcast")
            for tch in range(4):
                g_ps = psS.tile([128, NEXP], F32, tag="sums")
                for ec in range(NB):
                    nc.tensor.matmul(g_ps,
                                     lhsT=h2T[:, ec, tch * 128:(tch + 1) * 128],
                                     rhs=gwt_sb[:, ec, :],
                                     start=(ec == 0), stop=(ec == NB - 1))
                ge = sml.tile([128, NEXP], F32, tag="ge")
                nc.scalar.activation(out=ge, in_=g_ps, func=AF.Exp)
                gs = sml.tile([128, 1], F32, tag="gs")
                nc.vector.reduce_sum(gs, ge, axis=mybir.AxisListType.X)
                gr = sml.tile([128, 1], F32, tag="gr")
                nc.vector.reciprocal(gr, gs)
                gates = sml.tile([128, NEXP], F32, tag="gates")
                nc.vector.tensor_scalar_mul(gates, ge, gr)
                nc.tensor.matmul(p8_ps, lhsT=ones_col_f, rhs=gates,
                                 start=(tch == 0), stop=(tch == 3))
                top8 = sml.tile([128, NEXP], F32, tag="top8")
                nc.vector.max(out=top8, in_=gates)
                msk = sml.tile([128, NEXP], F32, tag="msk")
                nc.vector.tensor_scalar(out=msk, in0=gates, scalar1=top8[:, 1:2],
                                        scalar2=None, op0=AL.is_ge)
                nc.vector.tensor_mul(cw_sb[:, tch, :], gates, msk)
            p8_sb = sml.tile([1, NEXP], F32, tag="p8")
            nc.vector.tensor_copy(p8_sb, p8_ps)
            nc.sync.dma_start(out=probs8[:, :], in_=p8_sb)
            # cw rows per expert at partition 0 (M=1 transposes vs identity)
            cwT_sb = big.tile([1, NEXP, NTOK], F32, tag="vsb")
            for e in range(NEXP):
                ct_ps = psS.tile([1, NTOK], F32, tag="sums")
                for tch in range(4):
                    nc.tensor.matmul(ct_ps[:, tch * 128:(tch + 1) * 128],
                                     lhsT=cw_sb[:, tch, e:e + 1], rhs=id_f,
                                     start=True, stop=True)
                nc.vector.tensor_copy(cwT_sb[0:1, e, :], ct_ps)

            # ---- dense MoE as FF=8192 MLP with per-(token,expert) scaling ----
            moe_acc = big.tile([128, NB, NTOK], F32, tag="bigC")
            for panel in range(4):
                z = big.tile([128, 16, NTOK], F32R, tag="bigA")
                for strip in range(4):          # 4 strips of 512 hid per panel
                    gstrip = panel * 4 + strip
                    e = gstrip // 2
                    if gstrip % 2 == 0:
                        cwb = psB.tile([128, NTOK], F32, tag="bcast")
                        nc.tensor.matmul(cwb, lhsT=ones_row_f,
                                         rhs=cwT_sb[0:1, e, :], start=True, stop=True)
                    ps4 = [psM.tile([128, NTOK], F32, tag="main", name=f"psq{i}") for i in range(4)]
                    for ec in range(NB):
                        wt = wst.tile([128, 512], F32R, tag="wst")
                        eng = nc.sync if ec % 2 == 0 else nc.scalar
                        eng.dma_start(
                            out=wt, in_=w1T[ec * 128:(ec + 1) * 128,
                                            gstrip * 512:(gstrip + 1) * 512])
                        for q in range(4):
                            nc.tensor.matmul(ps4[q], lhsT=wt[:, q * 128:(q + 1) * 128],
                                             rhs=h2T[:, ec, :],
                                             start=(ec == 0), stop=(ec == NB - 1))
                    for q in range(4):
                        sg = tmp.tile([128, NTOK], F32, tag="scr")
                        nc.scalar.activation(out=sg, in_=ps4[q], func=AF.Sigmoid)
                        nc.vector.tensor_mul(sg, sg, ps4[q])
                        nc.vector.tensor_mul(z[:, strip * 4 + q, :], sg, cwb)
                # FFN2 over this panel: contract 16 hc
                for dpair in range(8):
                    ps2 = [psA.tile([128, NTOK], F32, tag="acc", name=f"psm{i}") for i in range(2)]
                    for hcp in range(16):
                        hc = panel * 16 + hcp
                        wt = wst.tile([128, 256], F32R, tag="wst")
                        eng = nc.sync if hc % 2 == 0 else nc.scalar
                        eng.dma_start(
                            out=wt, in_=w2T[hc * 128:(hc + 1) * 128,
                                            dpair * 256:(dpair + 1) * 256])
                        for j in range(2):
                            nc.tensor.matmul(ps2[j], lhsT=wt[:, j * 128:(j + 1) * 128],
                                             rhs=z[:, hcp, :],
                                             start=(hcp == 0), stop=(hcp == 15))
                    for j in range(2):
                        d = dpair * 2 + j
                        if panel == 0:
                            nc.vector.tensor_copy(moe_acc[:, d, :], ps2[j])
                        else:
                            nc.vector.tensor_add(moe_acc[:, d, :], moe_acc[:, d, :],
                                                 ps2[j])

            # ---- final residual + store ----
            for d in range(NB):
                ot = tmp.tile([128, NTOK], F32, tag="scr")
                nc.vector.tensor_add(ot, moe_acc[:, d, :], x1T[:, d, :])
                nc.sync.dma_start(out=outT[d * 128:(d + 1) * 128, :], in_=ot)

    nc.compile()
    return nc


def _host_prep(x, attention_mask, norm1_w, qkv_W, out_W, norm2_w, gate_W, W1, W2):
    """Build per-core input maps. attention_mask is all-ones per the spec."""
    x2 = np.asarray(x, np.float32).reshape(B * S, E)
    qkv_WT = np.ascontiguousarray(np.asarray(qkv_W, np.float32).T)
    out_WT = np.ascontiguousarray(np.asarray(out_W, np.float32).T)
    w1T = np.ascontiguousarray(np.asarray(W1, np.float32).reshape(HID, E).T)
    w2T = np.ascontiguousarray(
        np.asarray(W2, np.float32).transpose(0, 2, 1).reshape(HID, E))
    gate_WT = np.ascontiguousarray(np.asarray(gate_W, np.float32).T)
    n1w = np.ascontiguousarray(np.asarray(norm1_w, np.float32).reshape(NB, 128).T)
    n2w = np.ascontiguousarray(np.asarray(norm2_w, np.float32).reshape(NB, 128).T)
    identf = np.eye(128, dtype=np.float32)
    # additive causal mask in e-major permuted coords, transposed [tc', sc']
    i_q = np.arange(S) % 64
    e_q = np.arange(S) // 64
    allowed = (i_q[None, :][..., None] * 0)  # placeholder, built below
    iq = i_q[None, :]; eq = e_q[None, :]     # sc' axis
    it = i_q[:, None]; et = e_q[:, None]     # tc' axis
    allowed = (it < iq) | ((it == iq) & (et <= eq))
    maskT = np.where(allowed, 0.0, MASK_NEG).astype(np.float32)
    import ml_dtypes
    maskT = maskT.astype(ml_dtypes.bfloat16)

    shared = dict(qkv_WT=qkv_WT, out_WT=out_WT, w1T=w1T, w2T=w2T,
                  gate_WT=gate_WT, n1w=n1w, n2w=n2w, maskT=maskT, identf=identf)
    in_maps = []
    for c in range(N_CORES):
        xT = np.ascontiguousarray(x2[c * NTOK:(c + 1) * NTOK].T)
        in_maps.append(dict(shared, xT=xT))
    return in_maps


def kernel(x, attention_mask, norm1_w, qkv_W, out_W, norm2_w, gate_W, W1, W2):
    if "nc" not in _CACHED:
        _CACHED["nc"] = _build()
    nc = _CACHED["nc"]
    in_maps = _host_prep(x, attention_mask, norm1_w, qkv_W, out_W, norm2_w,
                         gate_W, W1, W2)
    res = run_bass_kernel_spmd(nc, in_maps, core_ids=list(range(N_CORES)),
                               **_CACHED.get("run_kwargs", {}))
    _CACHED["last_results"] = res
    out = np.empty((B * S, E), np.float32)
    probs_sum = np.zeros(NEXP, np.float64)
    for c in range(N_CORES):
        out[c * NTOK:(c + 1) * NTOK] = res.results[c]["outT"].T
        probs_sum += res.results[c]["probs8"][0].astype(np.float64)
    probs = (probs_sum / (B * S)).astype(np.float32)
    mean = probs.mean()
    var = ((probs - mean) ** 2).mean()
    loss = np.float32(var / (mean ** 2 + 1e-10))
    return out.reshape(B, S, E), loss
